# revision 24
# baseline (speedup 1.0000x reference)
"""Trainium2 Bass kernel for nn_MultiHeadAttention_7516192768586.

Full MHA: QKV projection -> masked softmax attention -> merge heads ->
residual add -> LayerNorm.  B=2, T=2048, D=1024, 16 heads (depth 64).

Sharding (8 cores): 2 batches x 4 head-groups (4 heads each, i.e. a
256-channel slice of the projected dims).  Each core computes attention for
its 4 heads, the residual+LayerNorm for its 256 output *columns* over all
2048 rows of its batch; LayerNorm statistics (sum x, sum x^2 over the full
1024 channels) are combined with a tiny 16KB AllReduce within each
batch-group of 4 cores.  Host reassembles the 8 (2048, 256) column slices.

Key host-side prep (inputs are free to reshape/re-layout when sharding):
  * q/k/v fed transposed (contraction dim D on partitions) as bf16
  * masked keys are compacted away on the host (exactly exp(PAD) == 0)
  * the reference's jnp.repeat(mask, h, axis=0) is batch-major while its
    head stacking is head-major, so attention row (head eta, batch beta)
    is masked by mask[eta // 8] -- replicated here faithfully
  * V's weight matrix gets one extra "ones" output channel per head so the
    attention-context matmul also produces the softmax denominators
  * V's bias is folded into the residual (ctx = sum attn (V+bv) = ctx0 +
    bv since attn rows sum to 1)
"""

import sys

if "/opt/trn_rl_repo" not in sys.path:
    sys.path.insert(0, "/opt/trn_rl_repo")

import contextlib

import ml_dtypes
import numpy as np

import bass_rust as _br
import concourse.bass as bass
import concourse.tile as tile
from concourse import mybir
from concourse.bass_utils import run_bass_kernel_spmd
from concourse.vector_clock import ScopedClock

F32 = mybir.dt.float32
BF16 = mybir.dt.bfloat16
BF = ml_dtypes.bfloat16

PAD = float(-(2**32) + 1)
NUM_HEADS = 16
LN_EPS = 1e-5
B, T, D = 2, 2048, 1024
DEPTH = D // NUM_HEADS  # 64
HPC = 4  # heads per core
DD = HPC * DEPTH  # 256 projected channels per core
DV = 1152  # v contraction dim padded: 1024 + ones row + zeros (9 k-tiles)
NKV = DV // 128  # 9
TT = T // 128  # 16 t-tiles
AluOp = mybir.AluOpType
Act = mybir.ActivationFunctionType


class _TC(tile.TileContext):
    """TileContext whose tail drain splits its sem waits across 1-wait NOPs
    (this walrus build rejects >1 sync wait on one instruction)."""

    def _drain_and_barrier(self, tick_clock, wait_clock):
        nc = self.nc
        drain_inst = nc.sync.drain()
        wait_clock.add_sem_waits(
            drain_inst.ins, ScopedClock({None: tick_clock.global_clock})
        )
        si = drain_inst.ins.sync_info
        waits = list(si.on_wait) if si is not None and si.on_wait else []
        if len(waits) > 1:
            si.on_wait = waits[:1]
            for i in range(1, len(waits)):
                extra = nc.sync.nop()
                extra.ins.sync_info = _br.SyncInfo(
                    on_wait=waits[i : i + 1], on_update=[]
                )
        nc.all_engine_barrier()
        popped = nc._tile_sem_poison_stack.pop()
        assert popped is self._sem_poison
        assert self.sems is not None
        nc.clear_and_free_semaphores(list(self.sems.allocated().values()))
        nc.all_engine_barrier()


def _split_multi_waits(nc):
    """Move extra sem waits (>1 per instruction) onto same-engine NOPs
    inserted immediately before the instruction."""
    f = nc.m.functions[0]
    cur_bb = nc.cur_bb
    for block in f.blocks:
        insts = list(block.instructions)
        if not any(
            i.sync_info is not None
            and i.sync_info.on_wait
            and len(i.sync_info.on_wait) > 1
            for i in insts
        ):
            continue
        new_list = []
        for inst in insts:
            si = inst.sync_info
            if si is not None and si.on_wait and len(si.on_wait) > 1:
                waits = list(si.on_wait)
                si.on_wait = waits[:1]
                for w in waits[1:]:
                    eng = nc.engines[inst.engine]
                    nop = eng.nop()
                    tail_bb = cur_bb.bb if hasattr(cur_bb, "bb") else cur_bb
                    tl = list(tail_bb.instructions)
                    assert tl and tl[-1].name == nop.ins.name
                    tail_bb.instructions = tl[:-1]
                    nop.ins.sync_info = _br.SyncInfo(on_wait=[w], on_update=[])
                    new_list.append(nop.ins)
            new_list.append(inst)
        block.instructions = new_list


def _build(SP):
    """Build the per-core Bass program. SP = padded compacted key count."""
    NS = SP // 128  # s-tiles
    NKCH = (SP + 511) // 512  # 512-wide chunks of SP for the K projection

    nc = bass.Bass("TRN2", target_bir_lowering=False, debug=False, num_devices=8)

    p = lambda name, shape, dt: nc.declare_dram_parameter(name, shape, dt, isOutput=False)
    qT = p("qT", [D, T], BF16)
    kT = p("kT", [D, SP], BF16)
    vT = p("vT", [DV, SP], BF16)
    wqT = p("wqT", [D, DD], BF16)
    wkT = p("wkT", [D, DD], BF16)
    wvT = p("wvT", [DV, HPC * (DEPTH + 1)], BF16)
    bq = p("bq", [128, 2], F32)
    bk = p("bk", [128, 2], F32)
    pb = p("padbias", [128, NS], F32)
    qres = p("qres", [T, DD], F32)
    gam = p("gamma", [1, DD], F32)
    bet = p("beta", [1, DD], F32)
    out = nc.declare_dram_parameter("out", [T, DD], F32, isOutput=True)

    with _TC(nc) as tc, contextlib.ExitStack() as ctx:
        singles = ctx.enter_context(tc.tile_pool(name="singles", bufs=1))
        persist = ctx.enter_context(tc.tile_pool(name="persist", bufs=1))
        work = ctx.enter_context(tc.tile_pool(name="work", bufs=4))
        dram = ctx.enter_context(tc.tile_pool(name="dram", bufs=1, space="DRAM"))

        # ---- constants / weights ----
        wv_sb = singles.tile([128, NKV, HPC * (DEPTH + 1)], BF16)
        nc.sync.dma_start(out=wv_sb[:], in_=wvT[:].rearrange("(kt p) c -> p kt c", p=128))
        wk_sb = singles.tile([128, 8, DD], BF16)
        nc.sync.dma_start(out=wk_sb[:], in_=wkT[:].rearrange("(kt p) c -> p kt c", p=128))
        wq_sb = singles.tile([128, 8, DD], BF16)
        nc.sync.dma_start(out=wq_sb[:], in_=wqT[:].rearrange("(kt p) c -> p kt c", p=128))
        bq_sb = singles.tile([128, 2], F32)
        nc.scalar.dma_start(out=bq_sb[:], in_=bq[:])
        bk_sb = singles.tile([128, 2], F32)
        nc.scalar.dma_start(out=bk_sb[:], in_=bk[:])
        pb_sb = singles.tile([128, NS], F32)
        nc.scalar.dma_start(out=pb_sb[:], in_=pb[:])
        gam_sb = singles.tile([128, DD], F32)
        g_ap = gam[:]
        nc.scalar.dma_start(
            out=gam_sb[:],
            in_=bass.AP(tensor=g_ap.tensor, offset=g_ap.offset, ap=[[0, 128], list(g_ap.ap[-1])]),
        )
        bet_sb = singles.tile([128, DD], F32)
        b_ap = bet[:]
        nc.scalar.dma_start(
            out=bet_sb[:],
            in_=bass.AP(tensor=b_ap.tensor, offset=b_ap.offset, ap=[[0, 128], list(b_ap.ap[-1])]),
        )
        eps_sb = singles.tile([128, 1], F32)
        nc.vector.memset(eps_sb[:], LN_EPS)

        # ---- persistent activations ----
        QT_sb = persist.tile([128, 2, T], BF16)  # [dd-in-tile, ddt, t]
        KT_sb = persist.tile([128, 2, SP], BF16)
        VH_sb = persist.tile([128, NS, HPC * (DEPTH + 1)], BF16)  # [s, st, head*65+c]
        ctxT_sb = persist.tile([128, HPC, T], BF16)  # rows 0..64 valid
        nc.gpsimd.memset(ctxT_sb[:], 0.0)
        ctxn_sb = persist.tile([128, HPC, TT, 128], BF16)  # transposed ctx
        x_sb = persist.tile([128, TT, DD], BF16)
        stats_sb = persist.tile([128, 2 * TT], F32)  # cols 0..15 sumx, 16..31 sumsq

        # ---- input streaming: all big input DMAs up front (sync/HWDGE),
        # in consumption order: wv/wk/wq weights, vT, kT, qT ----
        ain = ctx.enter_context(tc.tile_pool(name="ain", bufs=8))
        vin = ctx.enter_context(tc.tile_pool(name="vin", bufs=NKV))
        vts = []
        for kt in range(NKV):
            t_ = vin.tile([128, SP], BF16, tag="vin", name="vt")
            nc.sync.dma_start(out=t_[:], in_=vT[128 * kt : 128 * (kt + 1), :])
            vts.append(t_)
        kin = []
        for kt in range(8):
            t_ = ain.tile([128, SP], BF16, tag="kin", name="kin")
            nc.sync.dma_start(out=t_[:], in_=kT[128 * kt : 128 * (kt + 1), :])
            kin.append(t_)
        qin = []
        for kt in range(8):
            t_ = ain.tile([128, T], BF16, tag="qin", name="qin")
            nc.sync.dma_start(out=t_[:], in_=qT[128 * kt : 128 * (kt + 1), :])
            qin.append(t_)

        # ---- V projection (own PSUM scope, closed before attention) ----
        with tc.tile_pool(name="pv", bufs=2, space="PSUM") as pv:
            for st in range(NS):
                ps = pv.tile([128, HPC * (DEPTH + 1)], F32, tag="pv")
                for kt in range(NKV):
                    nc.tensor.matmul(
                        ps[:],
                        vts[kt][:, 128 * st : 128 * (st + 1)],
                        wv_sb[:, kt, :],
                        start=(kt == 0),
                        stop=(kt == NKV - 1),
                    )
                nc.vector.tensor_copy(VH_sb[:, st, :], ps[:])

        # ---- interleaved K/Q projections + attention + pipelined LN ----
        stats_dram = [dram.tile([128, TT], F32, name=f"std{i}") for i in range(2)]
        ar_dram = [dram.tile([128, TT], F32, name=f"ard{i}") for i in range(2)]
        mu = singles.tile([128, TT], F32)
        rstd = singles.tile([128, TT], F32)
        epool = ctx.enter_context(tc.tile_pool(name="epool", bufs=4))
        qres_in = ctx.enter_context(tc.tile_pool(name="qres_in", bufs=16))
        pqk = ctx.enter_context(tc.tile_pool(name="pqk", bufs=2, space="PSUM"))
        pscore = ctx.enter_context(tc.tile_pool(name="pscore", bufs=2, space="PSUM"))
        pctx = ctx.enter_context(tc.tile_pool(name="pctx", bufs=1, space="PSUM"))

        def k_proj(ddt):
            for sch in range(NKCH):
                w = min(512, SP - 512 * sch)
                ps = pqk.tile([128, 512], F32, tag="pqk", name="kps")
                for kt in range(8):
                    nc.tensor.matmul(
                        ps[:, :w],
                        wk_sb[:, kt, 128 * ddt : 128 * (ddt + 1)],
                        kin[kt][:, 512 * sch : 512 * sch + w],
                        start=(kt == 0),
                        stop=(kt == 7),
                    )
                nc.vector.tensor_scalar(
                    out=KT_sb[:, ddt, 512 * sch : 512 * sch + w],
                    in0=ps[:, :w],
                    scalar1=bk_sb[:, ddt : ddt + 1],
                    scalar2=None,
                    op0=AluOp.add,
                )

        def q_proj(ddt):
            for tch in range(4):
                ps = pqk.tile([128, 512], F32, tag="pqk", name="qps")
                for kt in range(8):
                    nc.tensor.matmul(
                        ps[:],
                        wq_sb[:, kt, 128 * ddt : 128 * (ddt + 1)],
                        qin[kt][:, 512 * tch : 512 * (tch + 1)],
                        start=(kt == 0),
                        stop=(kt == 7),
                    )
                nc.vector.tensor_scalar(
                    out=QT_sb[:, ddt, 512 * tch : 512 * (tch + 1)],
                    in0=ps[:],
                    scalar1=bq_sb[:, ddt : ddt + 1],
                    scalar2=None,
                    op0=AluOp.add,
                )

        def attn_head(tch, hd):
            t0 = 1024 * tch
            ddt, h = hd // 2, hd % 2
            r0 = DEPTH * h
            cps = pctx.tile([DEPTH + 1, 1024], F32, tag="c", name="cps")
            for st in range(NS):
                sps = pscore.tile([128, 1024], F32, tag="s", name="sps")
                for q2 in range(2):
                    nc.tensor.matmul(
                        sps[:, 512 * q2 : 512 * (q2 + 1)],
                        KT_sb[r0 : r0 + DEPTH, ddt, 128 * st : 128 * (st + 1)],
                        QT_sb[r0 : r0 + DEPTH, ddt, t0 + 512 * q2 : t0 + 512 * (q2 + 1)],
                        start=True,
                        stop=True,
                    )
                # PE warmer: keeps the HAM activity window dense so the
                # array stays at full clock while ACT paces the loop
                wps = pqk.tile([128, 512], F32, tag="pqk", name="wps")
                nc.tensor.matmul(
                    wps[:],
                    wq_sb[:, 0, 0:128],
                    QT_sb[:, 0, 0:512],
                    start=True,
                    stop=True,
                    skip_group_check=True,
                )
                e = epool.tile([128, 1024], BF16, tag="e")
                nc.scalar.activation(
                    out=e[:],
                    in_=sps[:],
                    func=Act.Exp,
                    bias=pb_sb[:, st : st + 1],
                    scale=0.125,
                )
                for q2 in range(2):
                    nc.tensor.matmul(
                        cps[:, 512 * q2 : 512 * (q2 + 1)],
                        VH_sb[:, st, (DEPTH + 1) * hd : (DEPTH + 1) * (hd + 1)],
                        e[:, 512 * q2 : 512 * (q2 + 1)],
                        start=(st == 0),
                        stop=(st == NS - 1),
                    )
            nc.vector.tensor_copy(ctxT_sb[0 : DEPTH + 1, hd, t0 : t0 + 1024], cps[:])
            nc.sync.dma_start_transpose(
                ctxn_sb[:, hd, 8 * tch : 8 * (tch + 1), :],
                ctxT_sb[:, hd, t0 : t0 + 1024],
            )

        def phase5(tch):
            # batched reciprocals of the softmax denominators for this half
            rinv_all = work.tile([128, HPC, 8], F32, tag="rinv", name=f"rinv{tch}")
            sums_ap = bass.AP(
                tensor=ctxn_sb.tensor,
                offset=ctxn_sb[:, 0, 8 * tch, DEPTH].offset,
                ap=[ctxn_sb.ap[0], [TT * 128, HPC], [128, 8], [1, 1]],
            )
            nc.vector.reciprocal(rinv_all[:], sums_ap)
            for tt in range(8 * tch, 8 * tch + 8):
                ctx_gather = bass.AP(
                    tensor=ctxn_sb.tensor,
                    offset=ctxn_sb[:, 0, tt, 0].offset,
                    ap=[ctxn_sb.ap[0], [TT * 128, HPC], [1, DEPTH]],
                )
                rinv_b = bass.AP(
                    tensor=rinv_all.tensor,
                    offset=rinv_all[:, 0, tt - 8 * tch].offset,
                    ap=[rinv_all.ap[0], [8, HPC], [0, DEPTH]],
                )
                qr = qres_tiles[tt]
                x1 = work.tile([128, DD], F32, tag="x1")
                nc.vector.tensor_tensor(out=x1[:], in0=ctx_gather, in1=rinv_b, op=AluOp.mult)
                nc.vector.tensor_tensor(out=x_sb[:, tt, :], in0=x1[:], in1=qr[:], op=AluOp.add)
                nc.vector.tensor_reduce(
                    out=stats_sb[:, tt : tt + 1], in_=x_sb[:, tt, :],
                    axis=mybir.AxisListType.X, op=AluOp.add,
                )
                sq = work.tile([128, DD], F32, tag="x1")
                nc.scalar.activation(
                    out=sq[:], in_=x_sb[:, tt, :], func=Act.Square,
                    accum_out=stats_sb[:, TT + tt : TT + tt + 1],
                )
            half = work.tile([128, TT], F32, tag="half", name=f"half{tch}")
            nc.vector.tensor_copy(half[:, 0:8], stats_sb[:, 8 * tch : 8 * tch + 8])
            nc.vector.tensor_copy(half[:, 8:16], stats_sb[:, TT + 8 * tch : TT + 8 * tch + 8])
            nc.sync.dma_start(out=stats_dram[tch][:], in_=half[:])
            nc.gpsimd.collective_compute(
                "AllReduce",
                AluOp.add,
                replica_groups=[[0, 1, 2, 3], [4, 5, 6, 7]],
                ins=[stats_dram[tch][:].opt()],
                outs=[ar_dram[tch][:].opt()],
            )

        def phase7(tch):
            gst = work.tile([128, TT], F32, tag="gst", name=f"gst{tch}")
            nc.sync.dma_start(out=gst[:], in_=ar_dram[tch][:])
            nc.vector.tensor_scalar(
                out=mu[:, 8 * tch : 8 * tch + 8], in0=gst[:, 0:8],
                scalar1=1.0 / D, scalar2=None, op0=AluOp.mult,
            )
            ex2 = work.tile([128, 8], F32, tag="ex2")
            nc.vector.tensor_scalar(
                out=ex2[:], in0=gst[:, 8:16], scalar1=1.0 / D, scalar2=None, op0=AluOp.mult
            )
            var = work.tile([128, 8], F32, tag="ex2")
            nc.vector.tensor_tensor(
                out=var[:], in0=mu[:, 8 * tch : 8 * tch + 8],
                in1=mu[:, 8 * tch : 8 * tch + 8], op=AluOp.mult,
            )
            nc.vector.tensor_tensor(out=var[:], in0=ex2[:], in1=var[:], op=AluOp.subtract)
            sd = work.tile([128, 8], F32, tag="ex2")
            nc.scalar.activation(out=sd[:], in_=var[:], func=Act.Sqrt, bias=eps_sb[:, 0:1], scale=1.0)
            nc.vector.reciprocal(rstd[:, 8 * tch : 8 * tch + 8], sd[:])
            for tt in range(8 * tch, 8 * tch + 8):
                xn = work.tile([128, DD], F32, tag="xn")
                nc.vector.tensor_scalar(
                    out=xn[:],
                    in0=x_sb[:, tt, :],
                    scalar1=mu[:, tt : tt + 1],
                    scalar2=rstd[:, tt : tt + 1],
                    op0=AluOp.subtract,
                    op1=AluOp.mult,
                )
                xg = work.tile([128, DD], F32, tag="xn")
                nc.vector.tensor_tensor(out=xg[:], in0=xn[:], in1=gam_sb[:], op=AluOp.mult)
                xo = work.tile([128, DD], F32, tag="xn")
                nc.vector.tensor_tensor(out=xo[:], in0=xg[:], in1=bet_sb[:], op=AluOp.add)
                nc.scalar.dma_start(out=out[128 * tt : 128 * (tt + 1), :], in_=xo[:])

        qres_tiles = []
        for tt in range(TT):
            qr = qres_in.tile([128, DD], F32, tag="qr", name="qr")
            nc.sync.dma_start(out=qr[:], in_=qres[128 * tt : 128 * (tt + 1), :])
            qres_tiles.append(qr)

        k_proj(0)
        q_proj(0)
        attn_head(0, 0)
        attn_head(0, 1)
        k_proj(1)
        q_proj(1)
        attn_head(0, 2)
        attn_head(0, 3)
        phase5(0)
        for hd in range(HPC):
            attn_head(1, hd)
        phase5(1)
        phase7(0)
        phase7(1)

    _split_multi_waits(nc)
    return nc


_CACHE = {}
_LAST_IN_MAPS = None


def kernel(q, k, v, mask, causality, edge_fea, wq, bq, wk, bk, wv, bv, gamma, beta):
    # NB: the reference masks attention row (head eta, batch beta) with
    # mask[eta // 8]; with 4 heads per core this is mask[hg // 2].
    q = np.asarray(q, np.float32)
    k = np.asarray(k, np.float32)
    v = np.asarray(v, np.float32)
    mask = np.asarray(mask)
    wq = np.asarray(wq, np.float32)
    bq = np.asarray(bq, np.float32)
    wk = np.asarray(wk, np.float32)
    bk = np.asarray(bk, np.float32)
    wv = np.asarray(wv, np.float32)
    bv = np.asarray(bv, np.float32)
    gamma = np.asarray(gamma, np.float32)
    beta = np.asarray(beta, np.float32)
    assert int(np.asarray(causality)) == 0

    keep = [np.flatnonzero(mask[g] == 0) for g in range(2)]
    slens = [len(kp) for kp in keep]
    SP = max(128, ((max(slens) + 127) // 128) * 128)

    qT = [np.ascontiguousarray(q[b].T).astype(BF) for b in range(2)]
    kTc, vTc = {}, {}
    for b in range(2):
        for g in range(2):
            kk = np.zeros((D, SP), BF)
            kk[:, : slens[g]] = k[b][keep[g]].T.astype(BF)
            kTc[b, g] = kk
            vv = np.zeros((DV, SP), BF)
            vv[:D, : slens[g]] = v[b][keep[g]].T.astype(BF)
            vv[D, :] = BF(1.0)
            vTc[b, g] = vv
    pbias = {}
    for g in range(2):
        a = np.zeros((SP,), np.float32)
        a[slens[g] :] = PAD
        # device layout: (128, NS) with [p, st] = pad[st*128 + p]
        pbias[g] = np.ascontiguousarray(a.reshape(-1, 128).T)

    in_maps = []
    for c in range(8):
        b, hg = c // 4, c % 4
        g = hg // 2
        c0 = hg * DD
        wvp = np.zeros((DV, HPC * (DEPTH + 1)), BF)
        for hh in range(HPC):
            wvp[:D, hh * (DEPTH + 1) : hh * (DEPTH + 1) + DEPTH] = (
                wv[c0 + hh * DEPTH : c0 + (hh + 1) * DEPTH].T.astype(BF)
            )
            wvp[D, hh * (DEPTH + 1) + DEPTH] = BF(1.0)
        in_maps.append(
            {
                "qT": qT[b],
                "kT": kTc[b, g],
                "vT": vTc[b, g],
                "wqT": np.ascontiguousarray(wq[c0 : c0 + DD].T).astype(BF),
                "wkT": np.ascontiguousarray(wk[c0 : c0 + DD].T).astype(BF),
                "wvT": wvp,
                "bq": np.ascontiguousarray(bq[c0 : c0 + DD].reshape(2, 128).T),
                "bk": np.ascontiguousarray(bk[c0 : c0 + DD].reshape(2, 128).T),
                "padbias": pbias[g],
                "qres": np.ascontiguousarray(q[b][:, c0 : c0 + DD] + bv[c0 : c0 + DD]),
                "gamma": np.ascontiguousarray(gamma[c0 : c0 + DD].reshape(1, DD)),
                "beta": np.ascontiguousarray(beta[c0 : c0 + DD].reshape(1, DD)),
            }
        )

    global _LAST_IN_MAPS
    _LAST_IN_MAPS = in_maps
    if SP not in _CACHE:
        _CACHE[SP] = _build(SP)
    nc = _CACHE[SP]

    res = run_bass_kernel_spmd(nc, in_maps, list(range(8))).results

    full = np.empty((B, T, D), np.float32)
    for c in range(8):
        b, hg = c // 4, c % 4
        full[b, :, hg * DD : (hg + 1) * DD] = res[c]["out"]
    return full


# revision 25
# speedup vs baseline: 1.0123x; 1.0123x over previous
"""Trainium2 Bass kernel for nn_MultiHeadAttention_7516192768586.

Full MHA: QKV projection -> masked softmax attention -> merge heads ->
residual add -> LayerNorm.  B=2, T=2048, D=1024, 16 heads (depth 64).

Sharding (8 cores): 2 batches x 4 head-groups (4 heads each, i.e. a
256-channel slice of the projected dims).  Each core computes attention for
its 4 heads, the residual+LayerNorm for its 256 output *columns* over all
2048 rows of its batch; LayerNorm statistics (sum x, sum x^2 over the full
1024 channels) are combined with a tiny 16KB AllReduce within each
batch-group of 4 cores.  Host reassembles the 8 (2048, 256) column slices.

Key host-side prep (inputs are free to reshape/re-layout when sharding):
  * q/k/v fed transposed (contraction dim D on partitions) as bf16
  * masked keys are compacted away on the host (exactly exp(PAD) == 0)
  * the reference's jnp.repeat(mask, h, axis=0) is batch-major while its
    head stacking is head-major, so attention row (head eta, batch beta)
    is masked by mask[eta // 8] -- replicated here faithfully
  * V's weight matrix gets one extra "ones" output channel per head so the
    attention-context matmul also produces the softmax denominators
  * V's bias is folded into the residual (ctx = sum attn (V+bv) = ctx0 +
    bv since attn rows sum to 1)
"""

import sys

if "/opt/trn_rl_repo" not in sys.path:
    sys.path.insert(0, "/opt/trn_rl_repo")

import contextlib

import ml_dtypes
import numpy as np

import bass_rust as _br
import concourse.bass as bass
import concourse.tile as tile
from concourse import mybir
from concourse.bass_utils import run_bass_kernel_spmd
from concourse.vector_clock import ScopedClock

F32 = mybir.dt.float32
BF16 = mybir.dt.bfloat16
BF = ml_dtypes.bfloat16

PAD = float(-(2**32) + 1)
NUM_HEADS = 16
LN_EPS = 1e-5
B, T, D = 2, 2048, 1024
DEPTH = D // NUM_HEADS  # 64
HPC = 4  # heads per core
DD = HPC * DEPTH  # 256 projected channels per core
DV = 1152  # v contraction dim padded: 1024 + ones row + zeros (9 k-tiles)
NKV = DV // 128  # 9
TT = T // 128  # 16 t-tiles
AluOp = mybir.AluOpType
Act = mybir.ActivationFunctionType


class _TC(tile.TileContext):
    """TileContext whose tail drain splits its sem waits across 1-wait NOPs
    (this walrus build rejects >1 sync wait on one instruction)."""

    def _drain_and_barrier(self, tick_clock, wait_clock):
        nc = self.nc
        drain_inst = nc.sync.drain()
        wait_clock.add_sem_waits(
            drain_inst.ins, ScopedClock({None: tick_clock.global_clock})
        )
        si = drain_inst.ins.sync_info
        waits = list(si.on_wait) if si is not None and si.on_wait else []
        if len(waits) > 1:
            si.on_wait = waits[:1]
            for i in range(1, len(waits)):
                extra = nc.sync.nop()
                extra.ins.sync_info = _br.SyncInfo(
                    on_wait=waits[i : i + 1], on_update=[]
                )
        nc.all_engine_barrier()
        popped = nc._tile_sem_poison_stack.pop()
        assert popped is self._sem_poison
        assert self.sems is not None
        nc.clear_and_free_semaphores(list(self.sems.allocated().values()))
        nc.all_engine_barrier()


def _split_multi_waits(nc):
    """Move extra sem waits (>1 per instruction) onto same-engine NOPs
    inserted immediately before the instruction."""
    f = nc.m.functions[0]
    cur_bb = nc.cur_bb
    for block in f.blocks:
        insts = list(block.instructions)
        if not any(
            i.sync_info is not None
            and i.sync_info.on_wait
            and len(i.sync_info.on_wait) > 1
            for i in insts
        ):
            continue
        new_list = []
        for inst in insts:
            si = inst.sync_info
            if si is not None and si.on_wait and len(si.on_wait) > 1:
                waits = list(si.on_wait)
                si.on_wait = waits[:1]
                for w in waits[1:]:
                    eng = nc.engines[inst.engine]
                    nop = eng.nop()
                    tail_bb = cur_bb.bb if hasattr(cur_bb, "bb") else cur_bb
                    tl = list(tail_bb.instructions)
                    assert tl and tl[-1].name == nop.ins.name
                    tail_bb.instructions = tl[:-1]
                    nop.ins.sync_info = _br.SyncInfo(on_wait=[w], on_update=[])
                    new_list.append(nop.ins)
            new_list.append(inst)
        block.instructions = new_list


def _build(SP):
    """Build the per-core Bass program. SP = padded compacted key count."""
    NS = SP // 128  # s-tiles
    NKCH = (SP + 511) // 512  # 512-wide chunks of SP for the K projection

    nc = bass.Bass("TRN2", target_bir_lowering=False, debug=False, num_devices=8)

    p = lambda name, shape, dt: nc.declare_dram_parameter(name, shape, dt, isOutput=False)
    qT = p("qT", [D, T], BF16)
    kT = p("kT", [D, SP], BF16)
    vT = p("vT", [DV, SP], BF16)
    wqT = p("wqT", [D, DD], BF16)
    wkT = p("wkT", [D, DD], BF16)
    wvT = p("wvT", [DV, HPC * (DEPTH + 1)], BF16)
    bq = p("bq", [128, 2], F32)
    bk = p("bk", [128, 2], F32)
    pb = p("padbias", [128, NS], F32)
    qres = p("qres", [T, DD], F32)
    gam = p("gamma", [1, DD], F32)
    bet = p("beta", [1, DD], F32)
    out = nc.declare_dram_parameter("out", [T, DD], F32, isOutput=True)

    with _TC(nc) as tc, contextlib.ExitStack() as ctx:
        singles = ctx.enter_context(tc.tile_pool(name="singles", bufs=1))
        persist = ctx.enter_context(tc.tile_pool(name="persist", bufs=1))
        work = ctx.enter_context(tc.tile_pool(name="work", bufs=4))
        dram = ctx.enter_context(tc.tile_pool(name="dram", bufs=1, space="DRAM"))

        # ---- constants / weights ----
        wv_sb = singles.tile([128, NKV, HPC * (DEPTH + 1)], BF16)
        nc.sync.dma_start(out=wv_sb[:], in_=wvT[:].rearrange("(kt p) c -> p kt c", p=128))
        wk_sb = singles.tile([128, 8, DD], BF16)
        nc.sync.dma_start(out=wk_sb[:], in_=wkT[:].rearrange("(kt p) c -> p kt c", p=128))
        wq_sb = singles.tile([128, 8, DD], BF16)
        nc.sync.dma_start(out=wq_sb[:], in_=wqT[:].rearrange("(kt p) c -> p kt c", p=128))
        bq_sb = singles.tile([128, 2], F32)
        nc.scalar.dma_start(out=bq_sb[:], in_=bq[:])
        bk_sb = singles.tile([128, 2], F32)
        nc.scalar.dma_start(out=bk_sb[:], in_=bk[:])
        pb_sb = singles.tile([128, NS], F32)
        nc.scalar.dma_start(out=pb_sb[:], in_=pb[:])
        gam_sb = singles.tile([128, DD], F32)
        g_ap = gam[:]
        nc.scalar.dma_start(
            out=gam_sb[:],
            in_=bass.AP(tensor=g_ap.tensor, offset=g_ap.offset, ap=[[0, 128], list(g_ap.ap[-1])]),
        )
        bet_sb = singles.tile([128, DD], F32)
        b_ap = bet[:]
        nc.scalar.dma_start(
            out=bet_sb[:],
            in_=bass.AP(tensor=b_ap.tensor, offset=b_ap.offset, ap=[[0, 128], list(b_ap.ap[-1])]),
        )
        eps_sb = singles.tile([128, 1], F32)
        nc.vector.memset(eps_sb[:], LN_EPS)

        # ---- persistent activations ----
        QT_sb = persist.tile([128, 2, T], BF16)  # [dd-in-tile, ddt, t]
        KT_sb = persist.tile([128, 2, SP], BF16)
        VH_sb = persist.tile([128, NS, HPC * (DEPTH + 1)], BF16)  # [s, st, head*65+c]
        ctxT_sb = persist.tile([128, HPC, T], BF16)  # rows 0..64 valid
        nc.gpsimd.memset(ctxT_sb[:], 0.0)
        ctxn_sb = persist.tile([128, HPC, TT, 128], BF16)  # transposed ctx
        x_sb = persist.tile([128, TT, DD], BF16)
        stats_sb = persist.tile([128, 2 * TT], F32)  # cols 0..15 sumx, 16..31 sumsq

        # ---- input streaming: all big input DMAs up front (sync/HWDGE),
        # in consumption order: wv/wk/wq weights, vT, kT, qT ----
        ain = ctx.enter_context(tc.tile_pool(name="ain", bufs=8))
        vin = ctx.enter_context(tc.tile_pool(name="vin", bufs=NKV))
        vts = []
        for kt in range(NKV):
            t_ = vin.tile([128, SP], BF16, tag="vin", name="vt")
            nc.sync.dma_start(out=t_[:], in_=vT[128 * kt : 128 * (kt + 1), :])
            vts.append(t_)
        kin = []
        for kt in range(8):
            t_ = ain.tile([128, SP], BF16, tag="kin", name="kin")
            nc.sync.dma_start(out=t_[:], in_=kT[128 * kt : 128 * (kt + 1), :])
            kin.append(t_)
        qin = []
        for kt in range(8):
            t_ = ain.tile([128, T], BF16, tag="qin", name="qin")
            nc.sync.dma_start(out=t_[:], in_=qT[128 * kt : 128 * (kt + 1), :])
            qin.append(t_)

        # ---- V projection (own PSUM scope, closed before attention) ----
        with tc.tile_pool(name="pv", bufs=2, space="PSUM") as pv:
            for st in range(NS):
                ps = pv.tile([128, HPC * (DEPTH + 1)], F32, tag="pv")
                for kt in range(NKV):
                    nc.tensor.matmul(
                        ps[:],
                        vts[kt][:, 128 * st : 128 * (st + 1)],
                        wv_sb[:, kt, :],
                        start=(kt == 0),
                        stop=(kt == NKV - 1),
                    )
                nc.vector.tensor_copy(VH_sb[:, st, :], ps[:])

        # ---- interleaved K/Q projections + attention + pipelined LN ----
        stats_dram = [dram.tile([128, TT], F32, name=f"std{i}") for i in range(2)]
        ar_dram = [dram.tile([128, TT], F32, name=f"ard{i}") for i in range(2)]
        mu = singles.tile([128, TT], F32)
        rstd = singles.tile([128, TT], F32)
        epool = ctx.enter_context(tc.tile_pool(name="epool", bufs=4))
        qres_in = ctx.enter_context(tc.tile_pool(name="qres_in", bufs=16))
        pqk = ctx.enter_context(tc.tile_pool(name="pqk", bufs=2, space="PSUM"))
        pscore = ctx.enter_context(tc.tile_pool(name="pscore", bufs=2, space="PSUM"))
        pctx = ctx.enter_context(tc.tile_pool(name="pctx", bufs=1, space="PSUM"))

        def k_proj(ddt):
            for sch in range(NKCH):
                w = min(512, SP - 512 * sch)
                ps = pqk.tile([128, 512], F32, tag="pqk", name="kps")
                for kt in range(8):
                    nc.tensor.matmul(
                        ps[:, :w],
                        wk_sb[:, kt, 128 * ddt : 128 * (ddt + 1)],
                        kin[kt][:, 512 * sch : 512 * sch + w],
                        start=(kt == 0),
                        stop=(kt == 7),
                    )
                nc.vector.tensor_scalar(
                    out=KT_sb[:, ddt, 512 * sch : 512 * sch + w],
                    in0=ps[:, :w],
                    scalar1=bk_sb[:, ddt : ddt + 1],
                    scalar2=None,
                    op0=AluOp.add,
                )

        def q_proj(ddt):
            for tch in range(4):
                ps = pqk.tile([128, 512], F32, tag="pqk", name="qps")
                for kt in range(8):
                    nc.tensor.matmul(
                        ps[:],
                        wq_sb[:, kt, 128 * ddt : 128 * (ddt + 1)],
                        qin[kt][:, 512 * tch : 512 * (tch + 1)],
                        start=(kt == 0),
                        stop=(kt == 7),
                    )
                nc.vector.tensor_scalar(
                    out=QT_sb[:, ddt, 512 * tch : 512 * (tch + 1)],
                    in0=ps[:],
                    scalar1=bq_sb[:, ddt : ddt + 1],
                    scalar2=None,
                    op0=AluOp.add,
                )

        def attn_head(tch, hd):
            t0 = 1024 * tch
            ddt, h = hd // 2, hd % 2
            r0 = DEPTH * h
            cps = pctx.tile([DEPTH + 1, 1024], F32, tag="c", name="cps")
            for st in range(NS):
                sps = pscore.tile([128, 1024], F32, tag="s", name="sps")
                for q2 in range(2):
                    nc.tensor.matmul(
                        sps[:, 512 * q2 : 512 * (q2 + 1)],
                        KT_sb[r0 : r0 + DEPTH, ddt, 128 * st : 128 * (st + 1)],
                        QT_sb[r0 : r0 + DEPTH, ddt, t0 + 512 * q2 : t0 + 512 * (q2 + 1)],
                        start=True,
                        stop=True,
                    )
                # PE warmer: keeps the HAM activity window dense so the
                # array stays at full clock while ACT paces the loop
                wps = pqk.tile([128, 256], F32, tag="pqk", name="wps")
                nc.tensor.matmul(
                    wps[:],
                    wq_sb[:, 0, 0:128],
                    QT_sb[:, 0, 0:256],
                    start=True,
                    stop=True,
                    skip_group_check=True,
                )
                e = epool.tile([128, 1024], BF16, tag="e")
                nc.scalar.activation(
                    out=e[:],
                    in_=sps[:],
                    func=Act.Exp,
                    bias=pb_sb[:, st : st + 1],
                    scale=0.125,
                )
                for q2 in range(2):
                    nc.tensor.matmul(
                        cps[:, 512 * q2 : 512 * (q2 + 1)],
                        VH_sb[:, st, (DEPTH + 1) * hd : (DEPTH + 1) * (hd + 1)],
                        e[:, 512 * q2 : 512 * (q2 + 1)],
                        start=(st == 0),
                        stop=(st == NS - 1),
                    )
            nc.vector.tensor_copy(ctxT_sb[0 : DEPTH + 1, hd, t0 : t0 + 1024], cps[:])
            nc.sync.dma_start_transpose(
                ctxn_sb[:, hd, 8 * tch : 8 * (tch + 1), :],
                ctxT_sb[:, hd, t0 : t0 + 1024],
            )

        def phase5(tch):
            # batched reciprocals of the softmax denominators for this half
            rinv_all = work.tile([128, HPC, 8], F32, tag="rinv", name=f"rinv{tch}")
            sums_ap = bass.AP(
                tensor=ctxn_sb.tensor,
                offset=ctxn_sb[:, 0, 8 * tch, DEPTH].offset,
                ap=[ctxn_sb.ap[0], [TT * 128, HPC], [128, 8], [1, 1]],
            )
            nc.vector.reciprocal(rinv_all[:], sums_ap)
            for tt in range(8 * tch, 8 * tch + 8):
                ctx_gather = bass.AP(
                    tensor=ctxn_sb.tensor,
                    offset=ctxn_sb[:, 0, tt, 0].offset,
                    ap=[ctxn_sb.ap[0], [TT * 128, HPC], [1, DEPTH]],
                )
                rinv_b = bass.AP(
                    tensor=rinv_all.tensor,
                    offset=rinv_all[:, 0, tt - 8 * tch].offset,
                    ap=[rinv_all.ap[0], [8, HPC], [0, DEPTH]],
                )
                qr = qres_tiles[tt]
                x1 = work.tile([128, DD], F32, tag="x1")
                nc.vector.tensor_tensor(out=x1[:], in0=ctx_gather, in1=rinv_b, op=AluOp.mult)
                nc.vector.tensor_tensor(out=x_sb[:, tt, :], in0=x1[:], in1=qr[:], op=AluOp.add)
                nc.vector.tensor_reduce(
                    out=stats_sb[:, tt : tt + 1], in_=x_sb[:, tt, :],
                    axis=mybir.AxisListType.X, op=AluOp.add,
                )
                sq = work.tile([128, DD], F32, tag="x1")
                nc.scalar.activation(
                    out=sq[:], in_=x_sb[:, tt, :], func=Act.Square,
                    accum_out=stats_sb[:, TT + tt : TT + tt + 1],
                )
            half = work.tile([128, TT], F32, tag="half", name=f"half{tch}")
            nc.vector.tensor_copy(half[:, 0:8], stats_sb[:, 8 * tch : 8 * tch + 8])
            nc.vector.tensor_copy(half[:, 8:16], stats_sb[:, TT + 8 * tch : TT + 8 * tch + 8])
            nc.sync.dma_start(out=stats_dram[tch][:], in_=half[:])
            nc.gpsimd.collective_compute(
                "AllReduce",
                AluOp.add,
                replica_groups=[[0, 1, 2, 3], [4, 5, 6, 7]],
                ins=[stats_dram[tch][:].opt()],
                outs=[ar_dram[tch][:].opt()],
            )

        def phase7(tch):
            gst = work.tile([128, TT], F32, tag="gst", name=f"gst{tch}")
            nc.sync.dma_start(out=gst[:], in_=ar_dram[tch][:])
            nc.vector.tensor_scalar(
                out=mu[:, 8 * tch : 8 * tch + 8], in0=gst[:, 0:8],
                scalar1=1.0 / D, scalar2=None, op0=AluOp.mult,
            )
            ex2 = work.tile([128, 8], F32, tag="ex2")
            nc.vector.tensor_scalar(
                out=ex2[:], in0=gst[:, 8:16], scalar1=1.0 / D, scalar2=None, op0=AluOp.mult
            )
            var = work.tile([128, 8], F32, tag="ex2")
            nc.vector.tensor_tensor(
                out=var[:], in0=mu[:, 8 * tch : 8 * tch + 8],
                in1=mu[:, 8 * tch : 8 * tch + 8], op=AluOp.mult,
            )
            nc.vector.tensor_tensor(out=var[:], in0=ex2[:], in1=var[:], op=AluOp.subtract)
            sd = work.tile([128, 8], F32, tag="ex2")
            nc.scalar.activation(out=sd[:], in_=var[:], func=Act.Sqrt, bias=eps_sb[:, 0:1], scale=1.0)
            nc.vector.reciprocal(rstd[:, 8 * tch : 8 * tch + 8], sd[:])
            for tt in range(8 * tch, 8 * tch + 8):
                xn = work.tile([128, DD], F32, tag="xn")
                nc.vector.tensor_scalar(
                    out=xn[:],
                    in0=x_sb[:, tt, :],
                    scalar1=mu[:, tt : tt + 1],
                    scalar2=rstd[:, tt : tt + 1],
                    op0=AluOp.subtract,
                    op1=AluOp.mult,
                )
                xg = work.tile([128, DD], F32, tag="xn")
                nc.vector.tensor_tensor(out=xg[:], in0=xn[:], in1=gam_sb[:], op=AluOp.mult)
                xo = work.tile([128, DD], F32, tag="xn")
                nc.vector.tensor_tensor(out=xo[:], in0=xg[:], in1=bet_sb[:], op=AluOp.add)
                nc.scalar.dma_start(out=out[128 * tt : 128 * (tt + 1), :], in_=xo[:])

        qres_tiles = []
        for tt in range(TT):
            qr = qres_in.tile([128, DD], F32, tag="qr", name="qr")
            nc.sync.dma_start(out=qr[:], in_=qres[128 * tt : 128 * (tt + 1), :])
            qres_tiles.append(qr)

        k_proj(0)
        q_proj(0)
        attn_head(0, 0)
        attn_head(0, 1)
        k_proj(1)
        q_proj(1)
        attn_head(0, 2)
        attn_head(0, 3)
        phase5(0)
        for hd in range(HPC):
            attn_head(1, hd)
        phase5(1)
        phase7(0)
        phase7(1)

    _split_multi_waits(nc)
    return nc


_CACHE = {}
_LAST_IN_MAPS = None


def kernel(q, k, v, mask, causality, edge_fea, wq, bq, wk, bk, wv, bv, gamma, beta):
    # NB: the reference masks attention row (head eta, batch beta) with
    # mask[eta // 8]; with 4 heads per core this is mask[hg // 2].
    q = np.asarray(q, np.float32)
    k = np.asarray(k, np.float32)
    v = np.asarray(v, np.float32)
    mask = np.asarray(mask)
    wq = np.asarray(wq, np.float32)
    bq = np.asarray(bq, np.float32)
    wk = np.asarray(wk, np.float32)
    bk = np.asarray(bk, np.float32)
    wv = np.asarray(wv, np.float32)
    bv = np.asarray(bv, np.float32)
    gamma = np.asarray(gamma, np.float32)
    beta = np.asarray(beta, np.float32)
    assert int(np.asarray(causality)) == 0

    keep = [np.flatnonzero(mask[g] == 0) for g in range(2)]
    slens = [len(kp) for kp in keep]
    SP = max(128, ((max(slens) + 127) // 128) * 128)

    qT = [np.ascontiguousarray(q[b].T).astype(BF) for b in range(2)]
    kTc, vTc = {}, {}
    for b in range(2):
        for g in range(2):
            kk = np.zeros((D, SP), BF)
            kk[:, : slens[g]] = k[b][keep[g]].T.astype(BF)
            kTc[b, g] = kk
            vv = np.zeros((DV, SP), BF)
            vv[:D, : slens[g]] = v[b][keep[g]].T.astype(BF)
            vv[D, :] = BF(1.0)
            vTc[b, g] = vv
    pbias = {}
    for g in range(2):
        a = np.zeros((SP,), np.float32)
        a[slens[g] :] = PAD
        # device layout: (128, NS) with [p, st] = pad[st*128 + p]
        pbias[g] = np.ascontiguousarray(a.reshape(-1, 128).T)

    in_maps = []
    for c in range(8):
        b, hg = c // 4, c % 4
        g = hg // 2
        c0 = hg * DD
        wvp = np.zeros((DV, HPC * (DEPTH + 1)), BF)
        for hh in range(HPC):
            wvp[:D, hh * (DEPTH + 1) : hh * (DEPTH + 1) + DEPTH] = (
                wv[c0 + hh * DEPTH : c0 + (hh + 1) * DEPTH].T.astype(BF)
            )
            wvp[D, hh * (DEPTH + 1) + DEPTH] = BF(1.0)
        in_maps.append(
            {
                "qT": qT[b],
                "kT": kTc[b, g],
                "vT": vTc[b, g],
                "wqT": np.ascontiguousarray(wq[c0 : c0 + DD].T).astype(BF),
                "wkT": np.ascontiguousarray(wk[c0 : c0 + DD].T).astype(BF),
                "wvT": wvp,
                "bq": np.ascontiguousarray(bq[c0 : c0 + DD].reshape(2, 128).T),
                "bk": np.ascontiguousarray(bk[c0 : c0 + DD].reshape(2, 128).T),
                "padbias": pbias[g],
                "qres": np.ascontiguousarray(q[b][:, c0 : c0 + DD] + bv[c0 : c0 + DD]),
                "gamma": np.ascontiguousarray(gamma[c0 : c0 + DD].reshape(1, DD)),
                "beta": np.ascontiguousarray(beta[c0 : c0 + DD].reshape(1, DD)),
            }
        )

    global _LAST_IN_MAPS
    _LAST_IN_MAPS = in_maps
    if SP not in _CACHE:
        _CACHE[SP] = _build(SP)
    nc = _CACHE[SP]

    res = run_bass_kernel_spmd(nc, in_maps, list(range(8))).results

    full = np.empty((B, T, D), np.float32)
    for c in range(8):
        b, hg = c // 4, c % 4
        full[b, :, hg * DD : (hg + 1) * DD] = res[c]["out"]
    return full
